# revision 5
# baseline (speedup 1.0000x reference)
"""ConvAttention Trainium2 kernel (Bass/Tile), self-contained.

Problem (hardcoded): B=2, S=4096, HID=1024, QKV=1024, H=16, HD=64,
depthwise conv (KH,KW)=(4,1) stride (4,1) VALID over (S,HD) -> T=1024.

reference:
  qh = split_heads(q @ Wq) * exp(-log_temp)
  kc = conv(split_heads(q @ Wk)); vc = conv(split_heads(q @ Wv))
  logits = qh @ kc^T ; att = softmax(logits) ; out = (att @ vc) @ Wlin
  returns (out [2,4096,1024], logits [2,16,4096,1024])

Sharding: 8 cores = 2 batches x 4 head-groups (4 heads / core).
Host folds exp(-log_temp) into Wq, regroups q[b] into xtg[i,k,t] =
q[b][4t+i,k] (tap-major transposed) so every device matmul operand is
contiguous, and sums the 4 per-head-group partial out projections.

Device per core:
  prologue: kcT[d,t], vc[t,d] via tap-scaled weights (conv folded into
            the projection matmuls), vc augmented with a ones column so
            the AV matmul also yields softmax row sums.
  main:     per (tap-residue i, t-half) block of 512 sequence rows:
            qhT -> logits[s,t] (straight to HBM from PSUM) and
            logitsT[t,s] -> exp on ACT -> attT -> avT accumulation ->
            transpose -> 1/sum scale on eviction -> out projection.
"""

import os
import sys

sys.path.insert(0, "/opt/trn_rl_repo")

import numpy as np

import concourse.bass as bass
import concourse.mybir as mybir
import concourse.tile as tile
from concourse import bacc
from concourse.bass_utils import run_bass_kernel_spmd
from concourse.masks import make_identity

F32 = mybir.dt.float32
P = 128
KC = 8          # 1024 hidden / 128
TAPS = 4
NH = 4          # heads per core
HD = 64
T = 1024        # conv output length
DQ = NH * HD    # 256: per-core projection width
HID = 1024
S = 4096
NCORES = 8

LAST_RESULT = None  # BassKernelResults of the most recent run (for test.py)


def _build_kernel():
    nc = bacc.Bacc("TRN2", target_bir_lowering=False, debug=False)

    xtg = nc.dram_tensor("xtg", [TAPS, HID, T], F32, kind="ExternalInput").ap()
    wq = nc.dram_tensor("wq", [HID, DQ], F32, kind="ExternalInput").ap()
    wk = nc.dram_tensor("wk", [HID, DQ], F32, kind="ExternalInput").ap()
    wv = nc.dram_tensor("wv", [HID, DQ], F32, kind="ExternalInput").ap()
    wrow = nc.dram_tensor("wrow", [TAPS, DQ], F32, kind="ExternalInput").ap()
    wlin = nc.dram_tensor("wlin", [DQ, HID], F32, kind="ExternalInput").ap()
    lg_out = nc.dram_tensor("logits_p", [NH, S, T], F32, kind="ExternalOutput").ap()
    o_out = nc.dram_tensor("out_p", [S, HID], F32, kind="ExternalOutput").ap()

    with tile.TileContext(nc) as tc:
        _body(tc, xtg, wq, wk, wv, wrow, wlin, lg_out, o_out)
    nc.compile()
    return nc


def _body(tc, xtg, wq, wk, wv, wrow, wlin, lg_out, o_out):
    from contextlib import ExitStack

    nc = tc.nc
    Exp = mybir.ActivationFunctionType.Exp
    Copy = mybir.ActivationFunctionType.Copy

    with ExitStack() as ctx:
        # ---------------- persistent tiles ----------------
        pers = ctx.enter_context(tc.tile_pool(name="pers", bufs=1))
        xtg_pool = ctx.enter_context(tc.tile_pool(name="xtgp", bufs=8))
        kct_pool = ctx.enter_context(tc.tile_pool(name="kctp", bufs=2))
        vc_pool = ctx.enter_context(tc.tile_pool(name="vcp", bufs=8))

        ident = pers.tile([P, P], F32, name="ident", tag="ident")
        make_identity(nc, ident)

        wq_sb = pers.tile([P, KC, DQ], F32, name="wq_sb", tag="wq")
        nc.sync.dma_start(out=wq_sb, in_=wq.rearrange("(c p) d -> p c d", p=P))
        wlin_sb = pers.tile([P, 2, HID], F32, name="wlin_sb", tag="wlin")
        nc.sync.dma_start(out=wlin_sb, in_=wlin.rearrange("(c p) h -> p c h", p=P))

        xs = []
        for c in range(KC):
            xt = xtg_pool.tile([P, TAPS, T], F32, name=f"xtg_sb{c}", tag="xtg")
            for i in range(TAPS):
                nc.sync.dma_start(
                    out=xt[:, i, :], in_=xtg[i, c * P : (c + 1) * P, :]
                )
            xs.append(xt)

        # kcT[pair]: [128 (2 heads x 64 d), 1024 t]
        kct = [kct_pool.tile([P, T], F32, name=f"kct{p}", tag="kct") for p in range(2)]
        # vc[tchunk]: [128 t, 4*65] (per head: 64 d cols + ones col for row sums)
        vcs = [
            vc_pool.tile([P, NH * 65], F32, name=f"vc{t_}", tag="vc") for t_ in range(8)
        ]
        for vt in vcs:
            nc.vector.memset(
                vt.rearrange("p (h x) -> p h x", h=NH)[:, :, 64:65], 1.0
            )

        # ---------------- prologue: kcT / vc ----------------
        with ExitStack() as pctx:
            wstream = pctx.enter_context(tc.tile_pool(name="wstream", bufs=2))
            wkst = pctx.enter_context(tc.tile_pool(name="wkst", bufs=2))
            cst2 = pctx.enter_context(tc.tile_pool(name="cst2", bufs=1))
            vct_pool = pctx.enter_context(tc.tile_pool(name="vctp", bufs=1))
            proj_ps = pctx.enter_context(
                tc.tile_pool(name="proj_ps", bufs=4, space="PSUM")
            )
            vctr_ps = pctx.enter_context(
                tc.tile_pool(name="vctr_ps", bufs=2, space="PSUM")
            )

            # conv weights broadcast across partitions: [128, 4 taps, 256]
            wbc = cst2.tile([P, TAPS, DQ], F32, name="wbc", tag="wbc")
            wrow_b = bass.AP(
                tensor=wrow.tensor, offset=wrow.offset, ap=[[0, P]] + list(wrow.ap)
            )
            nc.gpsimd.dma_start(out=wbc, in_=wrow_b)

            # pass 1: kcT  (accumulate 8 k-chunks x 4 taps)
            kps = [
                [proj_ps.tile([P, 512], F32, name="kcps", tag="pp") for _ in range(2)]
                for _ in range(2)
            ]
            for c in range(KC):
                wk_c = wstream.tile([P, DQ], F32, name="wk_c", tag="wkc")
                nc.sync.dma_start(out=wk_c, in_=wk[c * P : (c + 1) * P, :])
                wks = wkst.tile([P, TAPS, DQ], F32, name="wks", tag="wks")
                for i in range(TAPS):
                    nc.vector.tensor_mul(wks[:, i, :], wk_c, wbc[:, i, :])
                for pr in range(2):
                    for i in range(TAPS):
                        for hf in range(2):
                            nc.tensor.matmul(
                                kps[pr][hf],
                                wks[:, i, pr * P : (pr + 1) * P],
                                xs[c][:, i, hf * 512 : (hf + 1) * 512],
                                start=(c == 0 and i == 0),
                                stop=(c == KC - 1 and i == TAPS - 1),
                            )
            for pr in range(2):
                for hf in range(2):
                    nc.vector.tensor_copy(
                        out=kct[pr][:, hf * 512 : (hf + 1) * 512], in_=kps[pr][hf]
                    )

            # pass 2: vcT, then transpose into vc[t, d]
            for pr in range(2):
                vps = [
                    proj_ps.tile([P, 512], F32, name="vcps", tag="pp")
                    for _ in range(2)
                ]
                for c in range(KC):
                    wv_c = wstream.tile([P, DQ], F32, name="wv_c", tag="wkc")
                    nc.sync.dma_start(out=wv_c, in_=wv[c * P : (c + 1) * P, :])
                    wvs = wkst.tile([P, TAPS, DQ], F32, name="wvs", tag="wks")
                    for i in range(TAPS):
                        nc.vector.tensor_mul(wvs[:, i, :], wv_c, wbc[:, i, :])
                    for i in range(TAPS):
                        for hf in range(2):
                            nc.tensor.matmul(
                                vps[hf],
                                wvs[:, i, pr * P : (pr + 1) * P],
                                xs[c][:, i, hf * 512 : (hf + 1) * 512],
                                start=(c == 0 and i == 0),
                                stop=(c == KC - 1 and i == TAPS - 1),
                            )
                vct = vct_pool.tile([P, T], F32, name="vct", tag="vct")
                for hf in range(2):
                    nc.scalar.activation(
                        out=vct[:, hf * 512 : (hf + 1) * 512], in_=vps[hf], func=Copy
                    )
                for tch in range(8):
                    tp = vctr_ps.tile([P, P], F32, name="vtr", tag="vtr")
                    nc.tensor.transpose(tp, vct[:, tch * P : (tch + 1) * P], ident)
                    dst = vcs[tch][:, pr * 130 : pr * 130 + 130].rearrange(
                        "p (h x) -> p h x", h=2
                    )[:, :, 0:64]
                    nc.vector.tensor_copy(
                        out=dst, in_=tp.rearrange("p (h x) -> p h x", h=2)
                    )

        # ---------------- main loop ----------------
        with ExitStack() as mctx:
            qt_pool = mctx.enter_context(tc.tile_pool(name="qtp", bufs=2))
            att_pool = mctx.enter_context(tc.tile_pool(name="attp", bufs=4))
            ev_pool = mctx.enter_context(tc.tile_pool(name="evp", bufs=5))
            avs_pool = mctx.enter_context(tc.tile_pool(name="avsp", bufs=1))
            os_pool = mctx.enter_context(tc.tile_pool(name="osp", bufs=4))
            ot_pool = mctx.enter_context(tc.tile_pool(name="otp", bufs=2))
            rc_pool = mctx.enter_context(tc.tile_pool(name="rcp", bufs=2))
            mm_ps = mctx.enter_context(tc.tile_pool(name="mm_ps", bufs=2, space="PSUM"))
            lg_ps = mctx.enter_context(tc.tile_pool(name="lg_ps", bufs=2, space="PSUM"))
            lt_ps = mctx.enter_context(tc.tile_pool(name="lt_ps", bufs=2, space="PSUM"))
            avm_ps = mctx.enter_context(
                tc.tile_pool(name="avm_ps", bufs=1, space="PSUM")
            )
            avt_ps = mctx.enter_context(
                tc.tile_pool(name="avt_ps", bufs=1, space="PSUM")
            )

            # DRAM views with s = 4t + i split into (t, i)
            lgv = lg_out.rearrange("h (t four) c -> h t four c", four=TAPS)
            ov = o_out.rearrange("(t four) c -> t four c", four=TAPS)

            for blk in range(8):
                i, th = blk // 2, blk % 2
                t0 = th * 512

                # qhT for this block: [128 (2 heads x 64 d), 512 s]
                qts = []
                for pr in range(2):
                    qp = mm_ps.tile([P, 512], F32, name="qtps", tag="m")
                    for c in range(KC):
                        nc.tensor.matmul(
                            qp,
                            wq_sb[:, c, pr * P : (pr + 1) * P],
                            xs[c][:, i, t0 : t0 + 512],
                            start=(c == 0),
                            stop=(c == KC - 1),
                        )
                    qs = qt_pool.tile([P, 512], F32, name="qt_sb", tag="qt")
                    nc.vector.tensor_copy(out=qs, in_=qp)
                    qts.append(qs)

                osbs = [
                    os_pool.tile([P, DQ], F32, name=f"os{sj}", tag="os")
                    for sj in range(4)
                ]

                for h in range(NH):
                    pr, po = h // 2, (h % 2) * HD

                    # logitsT -> exp -> attT  (t on partitions, s free)
                    atts = []
                    for tch in range(8):
                        lt = lt_ps.tile([P, 512], F32, name="ltps", tag="lt")
                        nc.tensor.matmul(
                            lt,
                            kct[pr][po : po + HD, tch * P : (tch + 1) * P],
                            qts[pr][po : po + HD, :],
                            start=True,
                            stop=True,
                        )
                        at = att_pool.tile([P, 512], F32, name="att_sb", tag="att")
                        nc.scalar.activation(out=at, in_=lt, func=Exp)
                        atts.append(at)

                    # raw logits [s, t]: PSUM -> SBUF evict -> HBM
                    for sj in range(4):
                        for tH in range(2):
                            lg = lg_ps.tile([P, 512], F32, name="lgps", tag="lg")
                            nc.tensor.matmul(
                                lg,
                                qts[pr][po : po + HD, sj * P : (sj + 1) * P],
                                kct[pr][po : po + HD, tH * 512 : (tH + 1) * 512],
                                start=True,
                                stop=True,
                            )
                            lsb = ev_pool.tile([P, 512], F32, name="lg_sb", tag="ev")
                            if tH == 0:
                                nc.vector.tensor_copy(out=lsb, in_=lg)
                            else:
                                nc.scalar.activation(out=lsb, in_=lg, func=Copy)
                            nc.sync.dma_start(
                                out=lgv[
                                    h,
                                    t0 + sj * P : t0 + (sj + 1) * P,
                                    i,
                                    tH * 512 : (tH + 1) * 512,
                                ],
                                in_=lsb,
                            )

                    # avT[65, 512]: rows 0..63 = out^T, row 64 = softmax sums
                    avp = avt_ps.tile([65, 512], F32, name="avtps", tag="avt")
                    for tch in range(8):
                        nc.tensor.matmul(
                            avp,
                            vcs[tch][:, h * 65 : (h + 1) * 65],
                            atts[tch],
                            start=(tch == 0),
                            stop=(tch == 7),
                        )
                    avs = avs_pool.tile([65, 512], F32, name="avs_sb", tag="avs")
                    nc.scalar.activation(out=avs, in_=avp, func=Copy)

                    rct = rc_pool.tile([P, 4], F32, name="rct", tag="rc")
                    for sj in range(4):
                        avtr = avm_ps.tile([P, 65], F32, name="avmps", tag="avm")
                        nc.tensor.transpose(
                            avtr,
                            avs[0:65, sj * P : (sj + 1) * P],
                            ident[0:65, 0:65],
                        )
                        nc.vector.reciprocal(rct[:, sj : sj + 1], avtr[:, 64:65])
                        nc.scalar.activation(
                            out=osbs[sj][:, h * HD : (h + 1) * HD],
                            in_=avtr[:, 0:64],
                            func=Copy,
                            scale=rct[:, sj : sj + 1],
                        )

                # output projection per 128-row group
                for sj in range(4):
                    otp = mm_ps.tile([P, 2 * P], F32, name="otps", tag="m")
                    for dch in range(2):
                        nc.tensor.transpose(
                            otp[:, dch * P : (dch + 1) * P],
                            osbs[sj][:, dch * P : (dch + 1) * P],
                            ident,
                        )
                    ots = ot_pool.tile([P, 2 * P], F32, name="ot_sb", tag="ot")
                    nc.vector.tensor_copy(out=ots, in_=otp)
                    for hf in range(2):
                        op = mm_ps.tile([P, 512], F32, name="opps", tag="m")
                        for dch in range(2):
                            nc.tensor.matmul(
                                op,
                                ots[:, dch * P : (dch + 1) * P],
                                wlin_sb[:, dch, hf * 512 : (hf + 1) * 512],
                                start=(dch == 0),
                                stop=(dch == 1),
                            )
                        osb = ev_pool.tile([P, 512], F32, name="op_sb", tag="ev")
                        nc.vector.tensor_copy(out=osb, in_=op)
                        nc.sync.dma_start(
                            out=ov[
                                t0 + sj * P : t0 + (sj + 1) * P,
                                i,
                                hf * 512 : (hf + 1) * 512,
                            ],
                            in_=osb,
                        )


_NC_CACHE = None


def _get_nc():
    global _NC_CACHE
    if _NC_CACHE is None:
        _NC_CACHE = _build_kernel()
    return _NC_CACHE


def make_in_maps(q, Wq, Wk, Wv, Wlin, conv_w, log_temp):
    q = np.asarray(q, dtype=np.float32)
    Wq = np.asarray(Wq, dtype=np.float32)
    Wk = np.asarray(Wk, dtype=np.float32)
    Wv = np.asarray(Wv, dtype=np.float32)
    Wlin = np.asarray(Wlin, dtype=np.float32)
    conv_w = np.asarray(conv_w, dtype=np.float32)
    scale = float(np.exp(-np.asarray(log_temp, dtype=np.float64).reshape(-1)[0]))

    xtgs = []
    for b in range(2):
        # q[b][4t+i, k] -> xtg[i, k, t]
        xtgs.append(
            np.ascontiguousarray(q[b].reshape(T, TAPS, HID).transpose(1, 2, 0))
        )

    in_maps = []
    for core in range(NCORES):
        b, hg = core // 4, core % 4
        cs, ce = hg * DQ, (hg + 1) * DQ
        wrow = np.repeat(
            np.ascontiguousarray(conv_w[hg * NH : (hg + 1) * NH, 0, :, 0].T), HD, axis=1
        )  # [taps, 256]
        in_maps.append(
            {
                "xtg": xtgs[b],
                "wq": np.ascontiguousarray(Wq[:, cs:ce]) * np.float32(scale),
                "wk": np.ascontiguousarray(Wk[:, cs:ce]),
                "wv": np.ascontiguousarray(Wv[:, cs:ce]),
                "wrow": np.ascontiguousarray(wrow, dtype=np.float32),
                "wlin": np.ascontiguousarray(Wlin[cs:ce, :]),
            }
        )
    return in_maps


def kernel(q, Wq, Wk, Wv, Wlin, conv_w, log_temp):
    global LAST_RESULT
    nc = _get_nc()
    in_maps = make_in_maps(q, Wq, Wk, Wv, Wlin, conv_w, log_temp)
    trace = bool(os.environ.get("KERNEL_TRACE"))
    res = run_bass_kernel_spmd(
        nc, in_maps, core_ids=list(range(NCORES)), trace=trace
    )
    LAST_RESULT = res

    logits = np.empty((2, 16, S, T), dtype=np.float32)
    out = np.zeros((2, S, HID), dtype=np.float32)
    for core in range(NCORES):
        b, hg = core // 4, core % 4
        logits[b, hg * NH : (hg + 1) * NH] = res.results[core]["logits_p"]
        out[b] += res.results[core]["out_p"]
    return out, logits
